# revision 23
# baseline (speedup 1.0000x reference)
"""MoE-routing actor kernel for 8 Trainium2 NeuronCores.

Strategy (pure data parallel, expert-sorted, bf16 compute):
  - Host: for each expert m, deal its rows round-robin to the 8 cores so all
    cores get near-identical per-expert counts and can share ONE SPMD graph.
    Per-expert row capacities are the max count over cores (row-granular,
    ~0.3% padding); rows are packed sorted-by-expert.
  - The tiny shared trunk (fc1: 262144x32 @ 32x34, 0.6 GFLOP) plus relu runs
    on host BLAS; the device gets pre-packed transposed activations
    xaT [35, R] bf16 with an all-ones row 34 that folds the expert bias bout
    into the expert matmul.
  - Mask applied host-side: the device computes only the first A_DEV (<=128)
    kept output columns; masked columns are exact -1e9 filled host-side and
    kept columns beyond A_DEV (typically ~9) are computed on host BLAS.
  - Device, per 1024-row super-chunk: xaT [35,1024] load (gpsimd queue) ->
    transposed expert matmuls (stationary weff_e [35,A_DEV], moving xa run of
    <=512 rows, expert-boundary runs split) into a [A_DEV,1024] PSUM pair ->
    one [A_DEV,1024] f32->bf16 cast, alternating VectorE/ScalarE ->
    one 256KB contiguous store (sync queue).
  The device work is a single dense GEMM stream: ~1 PE cycle/row at the
  fixed 1.2 GHz PE clock, overlapped with casts and DMA.
"""

import os
import sys
from contextlib import ExitStack

sys.path.insert(0, "/opt/trn_rl_repo")

import numpy as np
import ml_dtypes

BF16 = ml_dtypes.bfloat16

B = 262144
NCORES = 8
J = 16
M = 12
H = 34
HP = H + 1  # fc1 output + ones row for bias folding
S_DIM = 32  # state dim
A = J * J  # 256 action logits
NEG = np.float32(-1.0e9)
SUPER = 1024  # rows per load/store chunk
HALF = 512  # PSUM-bank / matmul free-dim granule

_BUILD_CACHE: dict = {}
LAST_RESULT = None  # BassKernelResults of the most recent run (for profiling)


def _make_runs(caps, R):
    """Per 512-row half-chunk, the (expert, row0, row1) runs covering it."""
    offs = np.concatenate([[0], np.cumsum(caps)])
    assert offs[-1] == R
    runs = [[] for _ in range(R // HALF)]
    for m in range(len(caps)):
        lo, hi = int(offs[m]), int(offs[m + 1])
        if lo >= hi:
            continue
        for g in range(lo // HALF, (hi - 1) // HALF + 1):
            a = max(lo, g * HALF)
            b = min(hi, (g + 1) * HALF)
            if a < b:
                runs[g].append((m, a, b))
    return runs


def _build(R: int, caps: tuple, Adev: int) -> "object":
    import concourse.bass as bass
    import concourse.tile as tile
    from concourse import bacc, mybir

    f32 = mybir.dt.float32
    bf16 = mybir.dt.bfloat16
    nc = bacc.Bacc("TRN2", target_bir_lowering=False, debug=False)

    n_super = R // SUPER
    runs = _make_runs(list(caps), R)

    GRP = 3 if n_super % 3 == 0 else (2 if n_super % 2 == 0 else 1)
    xat_d = nc.declare_dram_parameter(
        "xat", [n_super // GRP, 2, HP, GRP * HALF], bf16, isOutput=False
    )
    weff_d = nc.declare_dram_parameter("weff", [HP, M * Adev], bf16, isOutput=False)
    n_pair = (n_super + 1) // 2
    out_d = nc.declare_dram_parameter(
        "out", [n_pair, Adev, 2 * SUPER], bf16, isOutput=True
    )

    with tile.TileContext(nc) as tc, ExitStack() as ctx:
        cpool = ctx.enter_context(tc.tile_pool(name="const", bufs=1))
        xapool = ctx.enter_context(tc.tile_pool(name="xa", bufs=6))
        outpool = ctx.enter_context(tc.tile_pool(name="outp", bufs=6))
        psopool = ctx.enter_context(
            tc.tile_pool(name="pso", bufs=4, space=bass.MemorySpace.PSUM)
        )

        # Weights duplicated at partition bases 0 and 64: consecutive 512-row
        # halves alternate PE row-strips, so each LDWEIGHTS targets strips
        # disjoint from the in-flight matmul and hides under it.
        weff = cpool.tile([64 + HP, M * Adev], bf16)
        nc.sync.dma_start(weff[0:HP, :], weff_d[:])
        nc.sync.dma_start(weff[64 : 64 + HP, :], weff_d[:])

        for sc in range(n_super):
            if sc % GRP == 0:
                # batched loads (GRP supers per DMA pair) on the dedicated
                # gpsimd queue: no head-of-line coupling, low issue overhead
                xa = xapool.tile([64 + HP, GRP * HALF], bf16)
                nc.gpsimd.dma_start(xa[0:HP, :], xat_d[sc // GRP, 0])
                nc.gpsimd.dma_start(xa[64 : 64 + HP, :], xat_d[sc // GRP, 1])
            j = sc % GRP

            if sc % 2 == 0:
                ot = outpool.tile([Adev, 2 * SUPER], bf16)
            pso = psopool.tile([Adev, SUPER], f32)
            for h in range(2):
                base = 0 if h == 0 else 64
                for (m, a, b) in runs[sc * 2 + h]:
                    c0 = a - sc * SUPER
                    c1 = b - sc * SUPER
                    nc.tensor.matmul(
                        pso[:, c0:c1],
                        weff[base : base + HP, m * Adev : (m + 1) * Adev],
                        xa[
                            base : base + HP,
                            j * HALF + c0 - h * HALF : j * HALF + c1 - h * HALF,
                        ],
                        start=True,
                        stop=True,
                    )
            # full-super casts, alternating engines (amortize the ~0.5us
            # fixed per-op cost); one store per 2 supers
            oslice = ot[:, (sc % 2) * SUPER : (sc % 2 + 1) * SUPER]
            if sc % 2 == 0:
                nc.vector.tensor_copy(oslice, pso[:])
            else:
                nc.scalar.copy(oslice, pso[:])
                nc.sync.dma_start(out_d[sc // 2], ot[:])
            if sc == n_super - 1 and n_super % 2 == 1:
                nc.sync.dma_start(out_d[sc // 2][:, 0:SUPER], ot[:, 0:SUPER])

    nc.compile()
    return nc


def kernel(states, epoch_idx, W1, b1, Wout, bout, mask):
    global LAST_RESULT
    from concourse.bass_utils import run_bass_kernel_spmd

    states = np.asarray(states, dtype=np.float32)
    epoch_idx = np.asarray(epoch_idx, dtype=np.int32)
    W1 = np.asarray(W1, dtype=np.float32)
    b1 = np.asarray(b1, dtype=np.float32)
    Wout = np.asarray(Wout, dtype=np.float32)
    bout = np.asarray(bout, dtype=np.float32)
    mask = np.asarray(mask, dtype=np.int32)

    keep = mask.reshape(A) != 0
    kept_cols = np.nonzero(keep)[0]
    Ak = int(len(kept_cols))
    if Ak == 0:
        return np.full((B, J, J), NEG, np.float32)
    Adev = min(Ak, 128)
    dev_cols = kept_cols[:Adev]
    rem_cols = kept_cols[Adev:]

    # --- shared trunk on host (tiny: ~0.6 GFLOP BLAS) ---
    x = np.maximum(states @ W1.T + b1[None, :], 0.0)  # [B, H] f32

    # --- route rows: per expert, deal round-robin across cores ---
    core_idx = [[None] * M for _ in range(NCORES)]
    for m in range(M):
        idx_m = np.nonzero(epoch_idx == m)[0]
        for i in range(NCORES):
            core_idx[i][m] = idx_m[i::NCORES]
    cnt = [[len(core_idx[i][m]) for m in range(M)] for i in range(NCORES)]
    # shared per-expert row capacity across cores (row-granular)
    caps = [max(cnt[i][m] for i in range(NCORES)) for m in range(M)]
    need = sum(caps)
    R = SUPER * ((max(need, B // NCORES) + SUPER - 1) // SUPER)
    caps[-1] += R - need  # dump slack into the last expert
    caps = tuple(caps)
    offs = np.concatenate([[0], np.cumsum(caps)])

    # --- effective expert weights (device columns only; bout in ones row) ---
    weff = np.zeros((HP, M * Adev), np.float32)
    for m in range(M):
        weff[:H, m * Adev : (m + 1) * Adev] = Wout[m][dev_cols].T
        weff[H, m * Adev : (m + 1) * Adev] = bout[m][dev_cols]
    weff_bf = weff.astype(BF16)

    # --- pack per-core transposed activations (bf16, super-chunk-major) ---
    in_maps = []
    for i in range(NCORES):
        packed = np.zeros((R, HP), np.float32)
        packed[:, H] = 1.0  # ones row for bias folding
        for m in range(M):
            r0 = int(offs[m])
            packed[r0 : r0 + cnt[i][m], :H] = x[core_idx[i][m]]
        n_super = R // SUPER
        GRP = 3 if n_super % 3 == 0 else (2 if n_super % 2 == 0 else 1)
        xat = np.ascontiguousarray(
            packed.astype(BF16)
            .reshape(n_super // GRP, GRP, 2, HALF, HP)
            .transpose(0, 2, 4, 1, 3)
            .reshape(n_super // GRP, 2, HP, GRP * HALF)
        )
        in_maps.append({"xat": xat, "weff": weff_bf})

    key = (R, caps, Adev)
    nc = _BUILD_CACHE.get(key)
    if nc is None:
        nc = _build(R, caps, Adev)
        _BUILD_CACHE[key] = nc

    res = run_bass_kernel_spmd(nc, in_maps, core_ids=list(range(NCORES)))
    LAST_RESULT = res

    # --- unpack: [n_pair, Adev, 2048] -> rows [R, Adev] ---
    out_kept = np.empty((B, Adev), np.float32)
    for i in range(NCORES):
        oc = np.asarray(res.results[i]["out"])
        rows = (
            oc.transpose(0, 2, 1).reshape(-1, Adev)[:R].astype(np.float32)
        )
        for m in range(M):
            r0 = int(offs[m])
            out_kept[core_idx[i][m]] = rows[r0 : r0 + cnt[i][m]]

    out_full = np.full((B, A), NEG, np.float32)
    out_full[:, dev_cols] = out_kept

    # --- host remainder: kept columns beyond the device's 128 ---
    if len(rem_cols):
        for m in range(M):
            rows_m = np.nonzero(epoch_idx == m)[0]
            out_full[rows_m[:, None], rem_cols[None, :]] = (
                x[rows_m] @ Wout[m][rem_cols].T + bout[m][rem_cols][None, :]
            )

    return out_full.reshape(B, J, J)
